# revision 9
# baseline (speedup 1.0000x reference)
"""ConcatCritic pair-grid MLP on 8 TRN2 NeuronCores — fp8 DoubleRow version.

Computes out[i, j] = f(x[i], y[j]) where f is a 3-hidden-layer MLP over the
concatenated pair, decomposed so the first layer is two small projections
summed by broadcast (no [B, B, A+B] concat tensor).

Sharding: the B^2 pair grid is split row-wise (x batch) across 8 cores;
y and all MLP parameters are replicated. Each core produces a [B/8, B]
score tile; the host concatenates them.

Precision scheme (validated in numpy AND on hw, rel err ~1.15e-2 vs 2e-2
gate):
- Input projections hx, hy exact fp32r, pre-scaled by S_H0 on the host.
- h0 = relu(hx + hy + b0) stored e4m3 at scale S_H0=4.
- Layers 1+2 are fp8 e4m3 DoubleRow matmuls (2 k-blocks of 128 per
  instruction): weights quantized at scale S_W=16 with an fp8 RESIDUAL
  tensor (W ~ Wq + Wr, both e4m3, summed in psum) cancelling most weight
  quantization noise. Scales are sqrt-balanced so every psum lands exactly
  on the next tensor's storage scale: psum1 = 64*z1 -> h1q stored at 64;
  psum2 = 1024*z2 -> h2 kept fp32 at 1024. NO rescale is needed anywhere
  on the device: every drain is a 2-op tensor_scalar (add bias, max 0) or
  an equivalent ScalarE activation(relu, bias, scale=1), so drains can go
  on any PSUM-capable engine (DVE/ACT; GPSIMD is SBUF-only and takes most
  of layer 0).
- Layer 3 uses the stationary-operand trick: lhsT = h2 128-pair block
  (fp32r), rhs = W3 k-column duplicated to 2 (fp32r ISA needs even moving
  size), out [128 pairs, 2] -- cost ~ moving size (2) per matmul, so L3 is
  nearly free on PE. k-blocks accumulate into one psum column pair per
  pair-block (single HW-exact psum group per bank). Final 1/1024 scale and
  +b3 on the host.

The emission is software-pipelined three stages deep (A: L0+L1 of tile
t+1, B: L2 of tile t, C: L3 of tile t-1) because engines execute their
queues in emission order: PE runs L1(t+1) while ACT/DVE drain L1(t).
"""

import numpy as np
import ml_dtypes

import concourse.bass as bass
import concourse.mybir as mybir
from concourse import bacc
from concourse.bass_utils import run_bass_kernel_spmd
from concourse.tile import TileContext

B = 256
A_DIM = 128
HID = 512
N_CORES = 8
ROWS = B // N_CORES  # 32 x-rows per core
KB = HID // 128  # 4 k-blocks of 128
PAIR_TILE = 512  # pairs per tile = 2 x-rows x 256 y-rows
ROWS_PER_TILE = PAIR_TILE // B  # 2
N_TILES = ROWS // ROWS_PER_TILE  # 16

S_H0 = 4.0  # h0 fp8 storage scale
S_W = 16.0  # weight fp8 quantization scale
S_H1 = S_H0 * S_W  # 64: psum1 scale == h1q storage scale (drain rescale-free)
S_PS2 = S_H1 * S_W  # 1024: psum2 / fp32-h2 scale, divided out on the host

RES1 = True  # W1 residual compensation
RES2 = True  # W2 residual compensation

# engine assignment tables ("P"=gpsimd, "V"=vector/DVE, "A"=scalar/ACT)
L0_ENG = ["P", "P", "P", "P", "P", "P", "V", "V"]  # per (k, a)
L1_DRAIN = ["A", "A", "A", "V"]  # per m-block
L2_DRAIN = ["A", "A", "V", "V"]  # per m-block

F32 = mybir.dt.float32
F32R = mybir.dt.float32r
F8 = mybir.dt.float8e4
E4 = ml_dtypes.float8_e4m3
DR = mybir.MatmulPerfMode.DoubleRow

_CACHE = {}


def _build_nc():
    nc = bacc.Bacc()

    xT = nc.declare_dram_parameter("xT", [A_DIM, ROWS], F32R, isOutput=False)
    yT = nc.declare_dram_parameter("yT", [A_DIM, B], F32R, isOutput=False)
    Wxs = nc.declare_dram_parameter("Wxs", [A_DIM, HID], F32R, isOutput=False)
    Wys = nc.declare_dram_parameter("Wys", [A_DIM, HID], F32R, isOutput=False)
    b0s = nc.declare_dram_parameter("b0s", [128, KB], F32, isOutput=False)
    W1q = nc.declare_dram_parameter("W1q", [128, KB, HID], F8, isOutput=False)
    W1r = nc.declare_dram_parameter("W1r", [128, KB, HID], F8, isOutput=False)
    W2q = nc.declare_dram_parameter("W2q", [128, KB, HID], F8, isOutput=False)
    W2r = nc.declare_dram_parameter("W2r", [128, KB, HID], F8, isOutput=False)
    b1s = nc.declare_dram_parameter("b1s", [128, KB], F32, isOutput=False)
    b2s = nc.declare_dram_parameter("b2s", [128, KB], F32, isOutput=False)
    W3c = nc.declare_dram_parameter("W3c", [128, KB, 2], F32R, isOutput=False)
    out = nc.declare_dram_parameter("out", [128, N_TILES * 4], F32, isOutput=True)

    relu = mybir.ActivationFunctionType.Relu
    ADD = mybir.AluOpType.add
    MAX = mybir.AluOpType.max

    with TileContext(nc) as tc:
        with (
            tc.tile_pool(name="const", bufs=1) as const,
            tc.tile_pool(name="work", bufs=3) as work,
            tc.tile_pool(name="sc_pool", bufs=2) as sc_pool,
            tc.tile_pool(name="ps1", bufs=4, space="PSUM") as ps1,
            tc.tile_pool(name="ps2", bufs=3, space="PSUM") as ps2,
            tc.tile_pool(name="ps3", bufs=1, space="PSUM") as ps3,
        ):
            # ---- load replicated constants -------------------------------
            xT_sb = const.tile([A_DIM, ROWS], F32R)
            yT_sb = const.tile([A_DIM, B], F32R)
            Wxs_sb = const.tile([A_DIM, HID], F32R)
            Wys_sb = const.tile([A_DIM, HID], F32R)
            b0_sb = const.tile([128, KB], F32)
            b1_sb = const.tile([128, KB], F32)
            b2_sb = const.tile([128, KB], F32)
            W1q_sb = const.tile([128, KB, HID], F8)
            W1r_sb = const.tile([128, KB, HID], F8)
            W2q_sb = const.tile([128, KB, HID], F8)
            W2r_sb = const.tile([128, KB, HID], F8)
            W3_sb = const.tile([128, KB, 2], F32R)

            nc.gpsimd.dma_start(xT_sb[:], xT[:, :])
            nc.gpsimd.dma_start(Wxs_sb[:], Wxs[:, :])
            nc.gpsimd.dma_start(yT_sb[:], yT[:, :])
            nc.gpsimd.dma_start(Wys_sb[:], Wys[:, :])
            nc.gpsimd.dma_start(b0_sb[:], b0s[:, :])
            nc.gpsimd.dma_start(W1q_sb[:], W1q[:, :, :])
            nc.gpsimd.dma_start(W1r_sb[:], W1r[:, :, :])
            nc.gpsimd.dma_start(b1_sb[:], b1s[:, :])
            nc.gpsimd.dma_start(W2q_sb[:], W2q[:, :, :])
            nc.gpsimd.dma_start(W2r_sb[:], W2r[:, :, :])
            nc.gpsimd.dma_start(b2_sb[:], b2s[:, :])
            nc.gpsimd.dma_start(W3_sb[:], W3c[:, :, :])

            ENG = {"P": nc.gpsimd, "V": nc.vector}

            def drain(which, dst, src, bias_col):
                if which == "A":
                    nc.scalar.activation(dst, src, relu, bias=bias_col, scale=1.0)
                else:
                    ENG[which].tensor_scalar(dst, src, bias_col, 0.0, ADD, MAX)

            # ---- input projections (exact fp32r, pre-scaled by S_H0) -----
            bxT = const.tile([128, KB, ROWS], F32)
            hyT = const.tile([128, KB, B], F32)
            for m in range(KB):
                sl = slice(m * 128, (m + 1) * 128)
                ph = ps1.tile([128, PAIR_TILE], F32, tag="ps1", name="ph")[:, :ROWS]
                nc.tensor.matmul(ph, Wxs_sb[:, sl], xT_sb[:], start=True, stop=True)
                nc.vector.tensor_scalar(
                    bxT[:, m], ph, b0_sb[:, m : m + 1], None, ADD
                )
                ph2 = ps2.tile([128, PAIR_TILE], F32, tag="ps2", name="ph2")[:, :B]
                nc.tensor.matmul(ph2, Wys_sb[:, sl], yT_sb[:], start=True, stop=True)
                nc.scalar.copy(out=hyT[:, m], in_=ph2)

            # ---- main pair-tile loop, 3-stage software pipeline ----------
            h1q_d = {}
            h2_d = {}
            sc_sb = None

            def stage_a(t):
                i0 = t * ROWS_PER_TILE
                # layer 0: h0q = e4m3(S_H0 * relu(hx_i + hy_j + b0))
                h0q = work.tile([128, KB, PAIR_TILE], F8, tag="h0")
                for k in range(KB):
                    for a in range(ROWS_PER_TILE):
                        ENG[L0_ENG[k * ROWS_PER_TILE + a]].tensor_scalar(
                            h0q[:, k, a * B : (a + 1) * B],
                            hyT[:, k],
                            bxT[:, k, i0 + a : i0 + a + 1],
                            0.0,
                            ADD,
                            MAX,
                        )
                # layer 1: fp8 DoubleRow + weight residual
                h1q = work.tile([128, KB, PAIR_TILE], F8, tag="h1")
                for m in range(KB):
                    mc = slice(m * 128, (m + 1) * 128)
                    pt = ps1.tile([128, PAIR_TILE], F32, tag="ps1", name="pt1")
                    nc.tensor.matmul(
                        pt, W1q_sb[:, 0:2, mc], h0q[:, 0:2, :],
                        start=True, stop=False, perf_mode=DR,
                    )
                    nc.tensor.matmul(
                        pt, W1q_sb[:, 2:4, mc], h0q[:, 2:4, :],
                        start=False, stop=not RES1, perf_mode=DR,
                    )
                    if RES1:
                        nc.tensor.matmul(
                            pt, W1r_sb[:, 0:2, mc], h0q[:, 0:2, :],
                            start=False, stop=False, perf_mode=DR,
                        )
                        nc.tensor.matmul(
                            pt, W1r_sb[:, 2:4, mc], h0q[:, 2:4, :],
                            start=False, stop=True, perf_mode=DR,
                        )
                    # psum1 = 64*z1; h1q = relu(psum1 + 64*b1), stored at 64
                    drain(L1_DRAIN[m], h1q[:, m], pt, b1_sb[:, m : m + 1])
                h1q_d[t] = h1q

            def stage_b(t):
                h1q = h1q_d.pop(t)
                # layer 2: fp8 DoubleRow + residual -> fp32 h2 at scale 1024
                h2 = work.tile([128, KB, PAIR_TILE], F32R, tag="h2")
                for m in range(KB):
                    mc = slice(m * 128, (m + 1) * 128)
                    pt = ps2.tile([128, PAIR_TILE], F32, tag="ps2", name="pt2")
                    nc.tensor.matmul(
                        pt, W2q_sb[:, 0:2, mc], h1q[:, 0:2, :],
                        start=True, stop=False, perf_mode=DR,
                    )
                    nc.tensor.matmul(
                        pt, W2q_sb[:, 2:4, mc], h1q[:, 2:4, :],
                        start=False, stop=not RES2, perf_mode=DR,
                    )
                    if RES2:
                        nc.tensor.matmul(
                            pt, W2r_sb[:, 0:2, mc], h1q[:, 0:2, :],
                            start=False, stop=False, perf_mode=DR,
                        )
                        nc.tensor.matmul(
                            pt, W2r_sb[:, 2:4, mc], h1q[:, 2:4, :],
                            start=False, stop=True, perf_mode=DR,
                        )
                    drain(L2_DRAIN[m], h2[:, m], pt, b2_sb[:, m : m + 1])
                h2_d[t] = h2

            def stage_c(t):
                nonlocal sc_sb
                h2 = h2_d.pop(t)
                # layer 3: stationary-h2 trick, one stage behind L2 so the
                # single ps3 bank never blocks PE.
                if t % 4 == 0:
                    sc_sb = sc_pool.tile([128, 16], F32, tag="sc")
                for pb in range(4):
                    pc = slice(pb * 128, (pb + 1) * 128)
                    p3 = ps3.tile([128, PAIR_TILE], F32, tag="ps3", name="p3")[:, :2]
                    for k in range(KB):
                        nc.tensor.matmul(
                            p3, h2[:, k, pc], W3_sb[:, k],
                            start=(k == 0), stop=(k == KB - 1),
                        )
                    col = (t % 4) * 4 + pb
                    nc.vector.tensor_scalar(
                        sc_sb[:, col : col + 1], p3[:, 0:1], 0.0, None, ADD
                    )
                if t % 4 == 3:
                    g = t // 4
                    nc.gpsimd.dma_start(out[:, g * 16 : (g + 1) * 16], sc_sb[:])

            stage_a(0)
            stage_a(1)
            stage_b(0)
            for t in range(2, N_TILES):
                stage_a(t)
                stage_b(t - 1)
                stage_c(t - 2)
            stage_b(N_TILES - 1)
            stage_c(N_TILES - 2)
            stage_c(N_TILES - 1)

    nc.compile()
    return nc


def _get_nc():
    if "nc" not in _CACHE:
        _CACHE["nc"] = _build_nc()
    return _CACHE["nc"]


def _q8(a):
    return np.clip(a, -240.0, 240.0).astype(E4)


def _prep_in_maps(inputs):
    f = lambda a: np.ascontiguousarray(np.asarray(a), dtype=np.float32)
    x, y = f(inputs["x"]), f(inputs["y"])
    W1, W2 = f(inputs["W1"]), f(inputs["W2"])

    def wq(W):
        # [HID, HID] -> quantized + residual, laid out [128, KB, HID]
        Ws = W * S_W
        q = _q8(Ws)
        r = _q8(Ws - q.astype(np.float32))
        re = lambda a: np.ascontiguousarray(
            a.reshape(KB, 128, HID).transpose(1, 0, 2)
        )
        return re(q), re(r)

    W1qa, W1ra = wq(W1)
    W2qa, W2ra = wq(W2)
    shared = {
        "yT": f(y.T),
        "Wxs": f(inputs["Wx"]) * np.float32(S_H0),
        "Wys": f(inputs["Wy"]) * np.float32(S_H0),
        "b0s": f(np.asarray(inputs["b0"]).reshape(KB, 128).T * S_H0),
        "W1q": W1qa,
        "W1r": W1ra,
        "W2q": W2qa,
        "W2r": W2ra,
        "b1s": f(np.asarray(inputs["b1"]).reshape(KB, 128).T * S_H1),
        "b2s": f(np.asarray(inputs["b2"]).reshape(KB, 128).T * S_PS2),
        "W3c": np.ascontiguousarray(
            np.repeat(
                f(np.asarray(inputs["W3"]).reshape(KB, 128).T)[:, :, None], 2, axis=2
            )
        ),
    }
    in_maps = []
    for m in range(N_CORES):
        im = dict(shared)
        im["xT"] = f(x[m * ROWS : (m + 1) * ROWS].T)
        in_maps.append(im)
    return in_maps


def run(trace=False, **inputs):
    nc = _get_nc()
    in_maps = _prep_in_maps(inputs)
    res = run_bass_kernel_spmd(nc, in_maps, core_ids=list(range(N_CORES)), trace=trace)
    b3 = np.float32(np.asarray(inputs["b3"]).reshape(-1)[0])
    inv = np.float32(1.0 / S_PS2)
    blocks = []
    for r in res.results:
        o = r["out"]  # [128, 64]: col = 4*t + pb, partition = pair-in-block
        a = o.reshape(128, N_TILES, 2, 2)  # [p, t, a_half, j_half]
        a = a.transpose(1, 2, 3, 0)  # [t, a_half, j_half, p]
        blocks.append(a.reshape(ROWS, B) * inv + b3)
    return np.concatenate(blocks, axis=0).astype(np.float32), res


def kernel(**inputs):
    out, _ = run(trace=False, **inputs)
    return out


# revision 11
# speedup vs baseline: 1.1066x; 1.1066x over previous
"""ConcatCritic pair-grid MLP on 8 TRN2 NeuronCores — fp8 DoubleRow version.

Computes out[i, j] = f(x[i], y[j]) where f is a 3-hidden-layer MLP over the
concatenated pair, decomposed so the first layer is two small projections
summed by broadcast (no [B, B, A+B] concat tensor).

Sharding: the B^2 pair grid is split row-wise (x batch) across 8 cores;
y and all MLP parameters are replicated. Each core produces a [B/8, B]
score tile; the host concatenates them.

Precision scheme (validated in numpy AND on hw, rel err ~1.15e-2 vs 2e-2
gate):
- Input projections hx, hy exact fp32r, pre-scaled by S_H0 on the host.
- h0 = relu(hx + hy + b0) stored e4m3 at scale S_H0=4.
- Layers 1+2 are fp8 e4m3 DoubleRow matmuls (2 k-blocks of 128 per
  instruction): weights quantized at scale S_W=16 with an fp8 RESIDUAL
  tensor (W ~ Wq + Wr, both e4m3, summed in psum) cancelling most weight
  quantization noise. Scales are sqrt-balanced so every psum lands exactly
  on the next tensor's storage scale: psum1 = 64*z1 -> h1q stored at 64;
  psum2 = 1024*z2 -> h2 kept fp32 at 1024. NO rescale is needed anywhere
  on the device: every drain is a 2-op tensor_scalar (add bias, max 0) or
  an equivalent ScalarE activation(relu, bias, scale=1), so drains can go
  on any PSUM-capable engine (DVE/ACT; GPSIMD is SBUF-only and takes most
  of layer 0).
- Layer 3 uses the stationary-operand trick: lhsT = h2 128-pair block
  (fp32r), rhs = W3 k-column duplicated to 2 (fp32r ISA needs even moving
  size), out [128 pairs, 2] -- cost ~ moving size (2) per matmul, so L3 is
  nearly free on PE. k-blocks accumulate into one psum column pair per
  pair-block (single HW-exact psum group per bank). Final 1/1024 scale and
  +b3 on the host.

The emission is software-pipelined three stages deep (A: L0+L1 of tile
t+1, B: L2 of tile t, C: L3 of tile t-1) because engines execute their
queues in emission order: PE runs L1(t+1) while ACT/DVE drain L1(t).
"""

import numpy as np
import ml_dtypes

import concourse.bass as bass
import concourse.mybir as mybir
from concourse import bacc
from concourse.bass_utils import run_bass_kernel_spmd
from concourse.tile import TileContext

B = 256
A_DIM = 128
HID = 512
N_CORES = 8
ROWS = B // N_CORES  # 32 x-rows per core
KB = HID // 128  # 4 k-blocks of 128
PAIR_TILE = 512  # pairs per tile = 2 x-rows x 256 y-rows
ROWS_PER_TILE = PAIR_TILE // B  # 2
N_TILES = ROWS // ROWS_PER_TILE  # 16

S_H0 = 4.0  # h0 fp8 storage scale
S_W = 16.0  # weight fp8 quantization scale
S_H1 = S_H0 * S_W  # 64: psum1 scale == h1q storage scale (drain rescale-free)
S_PS2 = S_H1 * S_W  # 1024: psum2 / fp32-h2 scale, divided out on the host

RES1 = True  # W1 residual compensation
RES2 = True  # W2 residual compensation

# engine assignment tables ("P"=gpsimd, "V"=vector/DVE, "A"=scalar/ACT)
L0_ENG = ["P", "P", "P", "P", "P", "P", "V", "V"]  # per (k, a)
L1_DRAIN = ["A", "A", "A", "V"]  # per m-block
L2_DRAIN = ["A", "A", "V", "V"]  # per m-block

F32 = mybir.dt.float32
F32R = mybir.dt.float32r
F8 = mybir.dt.float8e4
BF16 = mybir.dt.bfloat16
E4 = ml_dtypes.float8_e4m3
DR = mybir.MatmulPerfMode.DoubleRow

_CACHE = {}


def _build_nc():
    nc = bacc.Bacc()

    # packed loads: bf16 projection inputs [Wxs|Wys|yT], one small-f32 pack
    # [b0s|b1s|b2s|W3c], and the four fp8 weight tensors. Few big DMAs in
    # criticality order = short pipeline startup (SP issues a DMA every
    # ~600ns and the transfers serialize on the DMA engines).
    xTb = nc.declare_dram_parameter("xTb", [A_DIM, ROWS], BF16, isOutput=False)
    XYb = nc.declare_dram_parameter("XYb", [A_DIM, 2 * HID + B], BF16, isOutput=False)
    smalls = nc.declare_dram_parameter("smalls", [128, 3 * KB + 2 * KB], F32R, isOutput=False)
    W1q = nc.declare_dram_parameter("W1q", [128, KB, HID], F8, isOutput=False)
    W1r = nc.declare_dram_parameter("W1r", [128, KB, HID], F8, isOutput=False)
    W2q = nc.declare_dram_parameter("W2q", [128, KB, HID], F8, isOutput=False)
    W2r = nc.declare_dram_parameter("W2r", [128, KB, HID], F8, isOutput=False)
    out = nc.declare_dram_parameter("out", [128, N_TILES * 4], F32, isOutput=True)

    relu = mybir.ActivationFunctionType.Relu
    ADD = mybir.AluOpType.add
    MAX = mybir.AluOpType.max

    with TileContext(nc) as tc:
        with (
            tc.tile_pool(name="const", bufs=1) as const,
            tc.tile_pool(name="work", bufs=3) as work,
            tc.tile_pool(name="sc_pool", bufs=2) as sc_pool,
            tc.tile_pool(name="ps1", bufs=4, space="PSUM") as ps1,
            tc.tile_pool(name="ps2", bufs=3, space="PSUM") as ps2,
            tc.tile_pool(name="ps3", bufs=1, space="PSUM") as ps3,
        ):
            # ---- load replicated constants -------------------------------
            xT_sb = const.tile([A_DIM, ROWS], BF16)
            XY_sb = const.tile([A_DIM, 2 * HID + B], BF16)
            sm_sb = const.tile([128, 3 * KB + 2 * KB], F32R)
            W1q_sb = const.tile([128, KB, HID], F8)
            W1r_sb = const.tile([128, KB, HID], F8)
            W2q_sb = const.tile([128, KB, HID], F8)
            W2r_sb = const.tile([128, KB, HID], F8)

            nc.sync.dma_start(xT_sb[:], xTb[:, :])
            nc.sync.dma_start(XY_sb[:], XYb[:, :])
            nc.sync.dma_start(W1q_sb[:], W1q[:, :, :])
            nc.sync.dma_start(sm_sb[:], smalls[:, :])
            nc.sync.dma_start(W1r_sb[:], W1r[:, :, :])
            nc.sync.dma_start(W2q_sb[:], W2q[:, :, :])
            nc.sync.dma_start(W2r_sb[:], W2r[:, :, :])

            Wxs_sb = XY_sb[:, 0:HID]
            Wys_sb = XY_sb[:, HID : 2 * HID]
            yT_sb = XY_sb[:, 2 * HID : 2 * HID + B]
            b0_sb = sm_sb[:, 0:KB].bitcast(F32)
            b1_sb = sm_sb[:, KB : 2 * KB].bitcast(F32)
            b2_sb = sm_sb[:, 2 * KB : 3 * KB].bitcast(F32)
            W3_sb = sm_sb[:, 3 * KB : 5 * KB]

            ENG = {"P": nc.gpsimd, "V": nc.vector}

            def drain(which, dst, src, bias_col):
                if which == "A":
                    nc.scalar.activation(dst, src, relu, bias=bias_col, scale=1.0)
                else:
                    ENG[which].tensor_scalar(dst, src, bias_col, 0.0, ADD, MAX)

            # ---- input projections (exact fp32r, pre-scaled by S_H0) -----
            bxT = const.tile([128, KB, ROWS], F32)
            hyT = const.tile([128, KB, B], F32)
            for m in range(KB):
                sl = slice(m * 128, (m + 1) * 128)
                ph = ps1.tile([128, PAIR_TILE], F32, tag="ps1", name="ph")[:, :ROWS]
                nc.tensor.matmul(ph, Wxs_sb[:, sl], xT_sb[:], start=True, stop=True)
                nc.vector.tensor_scalar(
                    bxT[:, m], ph, b0_sb[:, m : m + 1], None, ADD
                )
                ph2 = ps2.tile([128, PAIR_TILE], F32, tag="ps2", name="ph2")[:, :B]
                nc.tensor.matmul(ph2, Wys_sb[:, sl], yT_sb[:], start=True, stop=True)
                nc.scalar.copy(out=hyT[:, m], in_=ph2)

            # ---- main pair-tile loop, 3-stage software pipeline ----------
            h1q_d = {}
            h2_d = {}
            sc_sb = None

            def stage_a(t):
                i0 = t * ROWS_PER_TILE
                # layer 0: h0q = e4m3(S_H0 * relu(hx_i + hy_j + b0))
                h0q = work.tile([128, KB, PAIR_TILE], F8, tag="h0")
                for k in range(KB):
                    for a in range(ROWS_PER_TILE):
                        ENG[L0_ENG[k * ROWS_PER_TILE + a]].tensor_scalar(
                            h0q[:, k, a * B : (a + 1) * B],
                            hyT[:, k],
                            bxT[:, k, i0 + a : i0 + a + 1],
                            0.0,
                            ADD,
                            MAX,
                        )
                # layer 1: fp8 DoubleRow + weight residual
                h1q = work.tile([128, KB, PAIR_TILE], F8, tag="h1")
                for m in range(KB):
                    mc = slice(m * 128, (m + 1) * 128)
                    pt = ps1.tile([128, PAIR_TILE], F32, tag="ps1", name="pt1")
                    nc.tensor.matmul(
                        pt, W1q_sb[:, 0:2, mc], h0q[:, 0:2, :],
                        start=True, stop=False, perf_mode=DR,
                    )
                    nc.tensor.matmul(
                        pt, W1q_sb[:, 2:4, mc], h0q[:, 2:4, :],
                        start=False, stop=not RES1, perf_mode=DR,
                    )
                    if RES1:
                        nc.tensor.matmul(
                            pt, W1r_sb[:, 0:2, mc], h0q[:, 0:2, :],
                            start=False, stop=False, perf_mode=DR,
                        )
                        nc.tensor.matmul(
                            pt, W1r_sb[:, 2:4, mc], h0q[:, 2:4, :],
                            start=False, stop=True, perf_mode=DR,
                        )
                    # psum1 = 64*z1; h1q = relu(psum1 + 64*b1), stored at 64
                    drain(L1_DRAIN[m], h1q[:, m], pt, b1_sb[:, m : m + 1])
                h1q_d[t] = h1q

            def stage_b(t):
                h1q = h1q_d.pop(t)
                # layer 2: fp8 DoubleRow + residual -> fp32 h2 at scale 1024
                h2 = work.tile([128, KB, PAIR_TILE], F32R, tag="h2")
                for m in range(KB):
                    mc = slice(m * 128, (m + 1) * 128)
                    pt = ps2.tile([128, PAIR_TILE], F32, tag="ps2", name="pt2")
                    nc.tensor.matmul(
                        pt, W2q_sb[:, 0:2, mc], h1q[:, 0:2, :],
                        start=True, stop=False, perf_mode=DR,
                    )
                    nc.tensor.matmul(
                        pt, W2q_sb[:, 2:4, mc], h1q[:, 2:4, :],
                        start=False, stop=not RES2, perf_mode=DR,
                    )
                    if RES2:
                        nc.tensor.matmul(
                            pt, W2r_sb[:, 0:2, mc], h1q[:, 0:2, :],
                            start=False, stop=False, perf_mode=DR,
                        )
                        nc.tensor.matmul(
                            pt, W2r_sb[:, 2:4, mc], h1q[:, 2:4, :],
                            start=False, stop=True, perf_mode=DR,
                        )
                    drain(L2_DRAIN[m], h2[:, m], pt, b2_sb[:, m : m + 1])
                h2_d[t] = h2

            def stage_c(t):
                nonlocal sc_sb
                h2 = h2_d.pop(t)
                # layer 3: stationary-h2 trick, one stage behind L2 so the
                # single ps3 bank never blocks PE.
                if t % 4 == 0:
                    sc_sb = sc_pool.tile([128, 16], F32, tag="sc")
                for pb in range(4):
                    pc = slice(pb * 128, (pb + 1) * 128)
                    p3 = ps3.tile([128, PAIR_TILE], F32, tag="ps3", name="p3")[:, :2]
                    for k in range(KB):
                        nc.tensor.matmul(
                            p3, h2[:, k, pc], W3_sb[:, 2 * k : 2 * k + 2],
                            start=(k == 0), stop=(k == KB - 1),
                        )
                    col = (t % 4) * 4 + pb
                    nc.vector.tensor_scalar(
                        sc_sb[:, col : col + 1], p3[:, 0:1], 0.0, None, ADD
                    )
                if t % 4 == 3:
                    g = t // 4
                    nc.gpsimd.dma_start(out[:, g * 16 : (g + 1) * 16], sc_sb[:])

            stage_a(0)
            stage_a(1)
            stage_b(0)
            for t in range(2, N_TILES):
                stage_a(t)
                stage_b(t - 1)
                stage_c(t - 2)
            stage_b(N_TILES - 1)
            stage_c(N_TILES - 2)
            stage_c(N_TILES - 1)

    nc.compile()
    return nc


def _get_nc():
    if "nc" not in _CACHE:
        _CACHE["nc"] = _build_nc()
    return _CACHE["nc"]


def _q8(a):
    return np.clip(a, -240.0, 240.0).astype(E4)


def _prep_in_maps(inputs):
    f = lambda a: np.ascontiguousarray(np.asarray(a), dtype=np.float32)
    x, y = f(inputs["x"]), f(inputs["y"])
    W1, W2 = f(inputs["W1"]), f(inputs["W2"])

    def wq(W):
        # [HID, HID] -> quantized + residual, laid out [128, KB, HID]
        Ws = W * S_W
        q = _q8(Ws)
        r = _q8(Ws - q.astype(np.float32))
        re = lambda a: np.ascontiguousarray(
            a.reshape(KB, 128, HID).transpose(1, 0, 2)
        )
        return re(q), re(r)

    W1qa, W1ra = wq(W1)
    W2qa, W2ra = wq(W2)
    BF = ml_dtypes.bfloat16
    xyb = np.concatenate(
        [f(inputs["Wx"]) * np.float32(S_H0), f(inputs["Wy"]) * np.float32(S_H0), f(y.T)],
        axis=1,
    ).astype(BF)
    w3c = np.repeat(
        f(np.asarray(inputs["W3"]).reshape(KB, 128).T)[:, :, None], 2, axis=2
    ).reshape(128, 2 * KB)
    sm = np.concatenate(
        [
            f(np.asarray(inputs["b0"]).reshape(KB, 128).T * S_H0),
            f(np.asarray(inputs["b1"]).reshape(KB, 128).T * S_H1),
            f(np.asarray(inputs["b2"]).reshape(KB, 128).T * S_PS2),
            w3c,
        ],
        axis=1,
    )
    shared = {
        "XYb": np.ascontiguousarray(xyb),
        "smalls": np.ascontiguousarray(sm, dtype=np.float32),
        "W1q": W1qa,
        "W1r": W1ra,
        "W2q": W2qa,
        "W2r": W2ra,
    }
    in_maps = []
    for m in range(N_CORES):
        im = dict(shared)
        im["xTb"] = np.ascontiguousarray(x[m * ROWS : (m + 1) * ROWS].T.astype(BF))
        in_maps.append(im)
    return in_maps


def run(trace=False, **inputs):
    nc = _get_nc()
    in_maps = _prep_in_maps(inputs)
    res = run_bass_kernel_spmd(nc, in_maps, core_ids=list(range(N_CORES)), trace=trace)
    b3 = np.float32(np.asarray(inputs["b3"]).reshape(-1)[0])
    inv = np.float32(1.0 / S_PS2)
    blocks = []
    for r in res.results:
        o = r["out"]  # [128, 64]: col = 4*t + pb, partition = pair-in-block
        a = o.reshape(128, N_TILES, 2, 2)  # [p, t, a_half, j_half]
        a = a.transpose(1, 2, 3, 0)  # [t, a_half, j_half, p]
        blocks.append(a.reshape(ROWS, B) * inv + b3)
    return np.concatenate(blocks, axis=0).astype(np.float32), res


def kernel(**inputs):
    out, _ = run(trace=False, **inputs)
    return out


# revision 12
# speedup vs baseline: 1.2779x; 1.1549x over previous
"""ConcatCritic pair-grid MLP on 8 TRN2 NeuronCores — fp8 DoubleRow version.

Computes out[i, j] = f(x[i], y[j]) where f is a 3-hidden-layer MLP over the
concatenated pair, decomposed so the first layer is two small projections
summed by broadcast (no [B, B, A+B] concat tensor).

Sharding: the B^2 pair grid is split row-wise (x batch) across 8 cores;
y and all MLP parameters are replicated. Each core produces a [B/8, B]
score tile; the host concatenates them.

Precision scheme (validated in numpy AND on hw, rel err ~1.15e-2 vs 2e-2
gate):
- Input projections hx, hy exact fp32r, pre-scaled by S_H0 on the host.
- h0 = relu(hx + hy + b0) stored e4m3 at scale S_H0=4.
- Layers 1+2 are fp8 e4m3 DoubleRow matmuls (2 k-blocks of 128 per
  instruction): weights quantized at scale S_W=16 with an fp8 RESIDUAL
  tensor (W ~ Wq + Wr, both e4m3, summed in psum) cancelling most weight
  quantization noise. Scales are sqrt-balanced so every psum lands exactly
  on the next tensor's storage scale: psum1 = 64*z1 -> h1q stored at 64;
  psum2 = 1024*z2 -> h2 kept fp32 at 1024. NO rescale is needed anywhere
  on the device: every drain is a 2-op tensor_scalar (add bias, max 0) or
  an equivalent ScalarE activation(relu, bias, scale=1), so drains can go
  on any PSUM-capable engine (DVE/ACT; GPSIMD is SBUF-only and takes most
  of layer 0).
- Layer 3 uses the stationary-operand trick: lhsT = h2 128-pair block
  (fp32r), rhs = W3 k-column duplicated to 2 (fp32r ISA needs even moving
  size), out [128 pairs, 2] -- cost ~ moving size (2) per matmul, so L3 is
  nearly free on PE. k-blocks accumulate into one psum column pair per
  pair-block (single HW-exact psum group per bank). Final 1/1024 scale and
  +b3 on the host.

The emission is software-pipelined three stages deep (A: L0+L1 of tile
t+1, B: L2 of tile t, C: L3 of tile t-1) because engines execute their
queues in emission order: PE runs L1(t+1) while ACT/DVE drain L1(t).
"""

import numpy as np
import ml_dtypes

import concourse.bass as bass
import concourse.mybir as mybir
from concourse import bacc
from concourse.bass_utils import run_bass_kernel_spmd
from concourse.tile import TileContext

B = 256
A_DIM = 128
HID = 512
N_CORES = 8
ROWS = B // N_CORES  # 32 x-rows per core
KB = HID // 128  # 4 k-blocks of 128
PAIR_TILE = 512  # pairs per tile = 2 x-rows x 256 y-rows
ROWS_PER_TILE = PAIR_TILE // B  # 2
N_TILES = ROWS // ROWS_PER_TILE  # 16

S_H0 = 4.0  # h0 fp8 storage scale
S_W = 16.0  # weight fp8 quantization scale
S_H1 = S_H0 * S_W  # 64: psum1 scale == h1q storage scale (drain rescale-free)
S_PS2 = S_H1 * S_W  # 1024: psum2 / fp32-h2 scale, divided out on the host

RES1 = True  # W1 residual compensation
RES2 = False  # W2 residual: not worth it (drain-bound, err 0.0138 < 2e-2 gate)

# engine assignment tables ("P"=gpsimd, "V"=vector/DVE, "A"=scalar/ACT)
L0_ENG = ["P", "P", "P", "P", "P", "P", "A", "V"]  # per (k, a)
L1_DRAIN = ["A", "A", "V", "V"]  # per m-block
L2_DRAIN = ["A", "A", "V", "V"]  # per m-block

F32 = mybir.dt.float32
F32R = mybir.dt.float32r
F8 = mybir.dt.float8e4
BF16 = mybir.dt.bfloat16
E4 = ml_dtypes.float8_e4m3
DR = mybir.MatmulPerfMode.DoubleRow

_CACHE = {}


def _build_nc():
    nc = bacc.Bacc()

    # packed loads: bf16 projection inputs [Wxs|Wys|yT], one small-f32 pack
    # [b0s|b1s|b2s|W3c], and the four fp8 weight tensors. Few big DMAs in
    # criticality order = short pipeline startup (SP issues a DMA every
    # ~600ns and the transfers serialize on the DMA engines).
    xTb = nc.declare_dram_parameter("xTb", [A_DIM, ROWS], BF16, isOutput=False)
    XYb = nc.declare_dram_parameter("XYb", [A_DIM, 2 * HID + B], BF16, isOutput=False)
    smalls = nc.declare_dram_parameter("smalls", [128, 3 * KB + 2 * KB], F32R, isOutput=False)
    W1q = nc.declare_dram_parameter("W1q", [128, KB, HID], F8, isOutput=False)
    W1r = nc.declare_dram_parameter("W1r", [128, KB, HID], F8, isOutput=False)
    W2q = nc.declare_dram_parameter("W2q", [128, KB, HID], F8, isOutput=False)
    W2r = nc.declare_dram_parameter("W2r", [128, KB, HID], F8, isOutput=False)
    out = nc.declare_dram_parameter("out", [128, N_TILES * 4], F32, isOutput=True)

    relu = mybir.ActivationFunctionType.Relu
    ADD = mybir.AluOpType.add
    MAX = mybir.AluOpType.max

    with TileContext(nc) as tc:
        with (
            tc.tile_pool(name="const", bufs=1) as const,
            tc.tile_pool(name="work", bufs=3) as work,
            tc.tile_pool(name="sc_pool", bufs=2) as sc_pool,
            tc.tile_pool(name="ps1", bufs=4, space="PSUM") as ps1,
            tc.tile_pool(name="ps2", bufs=3, space="PSUM") as ps2,
            tc.tile_pool(name="ps3", bufs=1, space="PSUM") as ps3,
        ):
            # ---- load replicated constants -------------------------------
            xT_sb = const.tile([A_DIM, ROWS], BF16)
            XY_sb = const.tile([A_DIM, 2 * HID + B], BF16)
            sm_sb = const.tile([128, 3 * KB + 2 * KB], F32R)
            W1q_sb = const.tile([128, KB, HID], F8)
            W1r_sb = const.tile([128, KB, HID], F8)
            W2q_sb = const.tile([128, KB, HID], F8)
            W2r_sb = const.tile([128, KB, HID], F8)

            nc.sync.dma_start(xT_sb[:], xTb[:, :])
            nc.sync.dma_start(XY_sb[:], XYb[:, :])
            nc.sync.dma_start(W1q_sb[:], W1q[:, :, :])
            nc.sync.dma_start(sm_sb[:], smalls[:, :])
            nc.sync.dma_start(W1r_sb[:], W1r[:, :, :])
            nc.sync.dma_start(W2q_sb[:], W2q[:, :, :])
            nc.sync.dma_start(W2r_sb[:], W2r[:, :, :])

            Wxs_sb = XY_sb[:, 0:HID]
            Wys_sb = XY_sb[:, HID : 2 * HID]
            yT_sb = XY_sb[:, 2 * HID : 2 * HID + B]
            b0_sb = sm_sb[:, 0:KB].bitcast(F32)
            b1_sb = sm_sb[:, KB : 2 * KB].bitcast(F32)
            b2_sb = sm_sb[:, 2 * KB : 3 * KB].bitcast(F32)
            W3_sb = sm_sb[:, 3 * KB : 5 * KB]

            ENG = {"P": nc.gpsimd, "V": nc.vector}

            def drain(which, dst, src, bias_col):
                if which == "A":
                    nc.scalar.activation(dst, src, relu, bias=bias_col, scale=1.0)
                else:
                    ENG[which].tensor_scalar(dst, src, bias_col, 0.0, ADD, MAX)

            # ---- input projections (exact fp32r, pre-scaled by S_H0) -----
            bxT = const.tile([128, KB, ROWS], F32)
            hyT = const.tile([128, KB, B], F32)
            for m in range(KB):
                sl = slice(m * 128, (m + 1) * 128)
                ph = ps1.tile([128, PAIR_TILE], F32, tag="ps1", name="ph")[:, :ROWS]
                nc.tensor.matmul(ph, Wxs_sb[:, sl], xT_sb[:], start=True, stop=True)
                nc.vector.tensor_scalar(
                    bxT[:, m], ph, b0_sb[:, m : m + 1], None, ADD
                )
                ph2 = ps2.tile([128, PAIR_TILE], F32, tag="ps2", name="ph2")[:, :B]
                nc.tensor.matmul(ph2, Wys_sb[:, sl], yT_sb[:], start=True, stop=True)
                nc.scalar.copy(out=hyT[:, m], in_=ph2)

            # ---- main pair-tile loop, 3-stage software pipeline ----------
            h1q_d = {}
            h2_d = {}
            sc_sb = None

            def stage_a(t):
                i0 = t * ROWS_PER_TILE
                # layer 0: h0q = e4m3(S_H0 * relu(hx_i + hy_j + b0))
                h0q = work.tile([128, KB, PAIR_TILE], F8, tag="h0")
                for k in range(KB):
                    for a in range(ROWS_PER_TILE):
                        which = L0_ENG[k * ROWS_PER_TILE + a]
                        if which == "A":
                            nc.scalar.activation(
                                h0q[:, k, a * B : (a + 1) * B],
                                hyT[:, k],
                                relu,
                                bias=bxT[:, k, i0 + a : i0 + a + 1],
                                scale=1.0,
                            )
                        else:
                            ENG[which].tensor_scalar(
                                h0q[:, k, a * B : (a + 1) * B],
                                hyT[:, k],
                                bxT[:, k, i0 + a : i0 + a + 1],
                                0.0,
                                ADD,
                                MAX,
                            )
                # layer 1: fp8 DoubleRow + weight residual
                h1q = work.tile([128, KB, PAIR_TILE], F8, tag="h1")
                for m in range(KB):
                    mc = slice(m * 128, (m + 1) * 128)
                    pt = ps1.tile([128, PAIR_TILE], F32, tag="ps1", name="pt1")
                    nc.tensor.matmul(
                        pt, W1q_sb[:, 0:2, mc], h0q[:, 0:2, :],
                        start=True, stop=False, perf_mode=DR,
                    )
                    nc.tensor.matmul(
                        pt, W1q_sb[:, 2:4, mc], h0q[:, 2:4, :],
                        start=False, stop=not RES1, perf_mode=DR,
                    )
                    if RES1:
                        nc.tensor.matmul(
                            pt, W1r_sb[:, 0:2, mc], h0q[:, 0:2, :],
                            start=False, stop=False, perf_mode=DR,
                        )
                        nc.tensor.matmul(
                            pt, W1r_sb[:, 2:4, mc], h0q[:, 2:4, :],
                            start=False, stop=True, perf_mode=DR,
                        )
                    # psum1 = 64*z1; h1q = relu(psum1 + 64*b1), stored at 64
                    drain(L1_DRAIN[m], h1q[:, m], pt, b1_sb[:, m : m + 1])
                h1q_d[t] = h1q

            def stage_b(t):
                h1q = h1q_d.pop(t)
                # layer 2: fp8 DoubleRow + residual -> fp32 h2 at scale 1024
                h2 = work.tile([128, KB, PAIR_TILE], F32R, tag="h2")
                for m in range(KB):
                    mc = slice(m * 128, (m + 1) * 128)
                    pt = ps2.tile([128, PAIR_TILE], F32, tag="ps2", name="pt2")
                    nc.tensor.matmul(
                        pt, W2q_sb[:, 0:2, mc], h1q[:, 0:2, :],
                        start=True, stop=False, perf_mode=DR,
                    )
                    nc.tensor.matmul(
                        pt, W2q_sb[:, 2:4, mc], h1q[:, 2:4, :],
                        start=False, stop=not RES2, perf_mode=DR,
                    )
                    if RES2:
                        nc.tensor.matmul(
                            pt, W2r_sb[:, 0:2, mc], h1q[:, 0:2, :],
                            start=False, stop=False, perf_mode=DR,
                        )
                        nc.tensor.matmul(
                            pt, W2r_sb[:, 2:4, mc], h1q[:, 2:4, :],
                            start=False, stop=True, perf_mode=DR,
                        )
                    drain(L2_DRAIN[m], h2[:, m], pt, b2_sb[:, m : m + 1])
                h2_d[t] = h2

            def stage_c(t):
                nonlocal sc_sb
                h2 = h2_d.pop(t)
                # layer 3: stationary-h2 trick, one stage behind L2 so the
                # single ps3 bank never blocks PE.
                if t % 4 == 0:
                    sc_sb = sc_pool.tile([128, 16], F32, tag="sc")
                for pb in range(4):
                    pc = slice(pb * 128, (pb + 1) * 128)
                    p3 = ps3.tile([128, PAIR_TILE], F32, tag="ps3", name="p3")[:, :2]
                    for k in range(KB):
                        nc.tensor.matmul(
                            p3, h2[:, k, pc], W3_sb[:, 2 * k : 2 * k + 2],
                            start=(k == 0), stop=(k == KB - 1),
                        )
                    col = (t % 4) * 4 + pb
                    nc.vector.tensor_scalar(
                        sc_sb[:, col : col + 1], p3[:, 0:1], 0.0, None, ADD
                    )
                if t % 4 == 3:
                    g = t // 4
                    nc.gpsimd.dma_start(out[:, g * 16 : (g + 1) * 16], sc_sb[:])

            stage_a(0)
            stage_a(1)
            stage_b(0)
            for t in range(2, N_TILES):
                stage_a(t)
                stage_b(t - 1)
                stage_c(t - 2)
            stage_b(N_TILES - 1)
            stage_c(N_TILES - 2)
            stage_c(N_TILES - 1)

    nc.compile()
    return nc


def _get_nc():
    if "nc" not in _CACHE:
        _CACHE["nc"] = _build_nc()
    return _CACHE["nc"]


def _q8(a):
    return np.clip(a, -240.0, 240.0).astype(E4)


def _prep_in_maps(inputs):
    f = lambda a: np.ascontiguousarray(np.asarray(a), dtype=np.float32)
    x, y = f(inputs["x"]), f(inputs["y"])
    W1, W2 = f(inputs["W1"]), f(inputs["W2"])

    def wq(W):
        # [HID, HID] -> quantized + residual, laid out [128, KB, HID]
        Ws = W * S_W
        q = _q8(Ws)
        r = _q8(Ws - q.astype(np.float32))
        re = lambda a: np.ascontiguousarray(
            a.reshape(KB, 128, HID).transpose(1, 0, 2)
        )
        return re(q), re(r)

    W1qa, W1ra = wq(W1)
    W2qa, W2ra = wq(W2)
    BF = ml_dtypes.bfloat16
    xyb = np.concatenate(
        [f(inputs["Wx"]) * np.float32(S_H0), f(inputs["Wy"]) * np.float32(S_H0), f(y.T)],
        axis=1,
    ).astype(BF)
    w3c = np.repeat(
        f(np.asarray(inputs["W3"]).reshape(KB, 128).T)[:, :, None], 2, axis=2
    ).reshape(128, 2 * KB)
    sm = np.concatenate(
        [
            f(np.asarray(inputs["b0"]).reshape(KB, 128).T * S_H0),
            f(np.asarray(inputs["b1"]).reshape(KB, 128).T * S_H1),
            f(np.asarray(inputs["b2"]).reshape(KB, 128).T * S_PS2),
            w3c,
        ],
        axis=1,
    )
    shared = {
        "XYb": np.ascontiguousarray(xyb),
        "smalls": np.ascontiguousarray(sm, dtype=np.float32),
        "W1q": W1qa,
        "W1r": W1ra,
        "W2q": W2qa,
        "W2r": W2ra,
    }
    in_maps = []
    for m in range(N_CORES):
        im = dict(shared)
        im["xTb"] = np.ascontiguousarray(x[m * ROWS : (m + 1) * ROWS].T.astype(BF))
        in_maps.append(im)
    return in_maps


def run(trace=False, **inputs):
    nc = _get_nc()
    in_maps = _prep_in_maps(inputs)
    res = run_bass_kernel_spmd(nc, in_maps, core_ids=list(range(N_CORES)), trace=trace)
    b3 = np.float32(np.asarray(inputs["b3"]).reshape(-1)[0])
    inv = np.float32(1.0 / S_PS2)
    blocks = []
    for r in res.results:
        o = r["out"]  # [128, 64]: col = 4*t + pb, partition = pair-in-block
        a = o.reshape(128, N_TILES, 2, 2)  # [p, t, a_half, j_half]
        a = a.transpose(1, 2, 3, 0)  # [t, a_half, j_half, p]
        blocks.append(a.reshape(ROWS, B) * inv + b3)
    return np.concatenate(blocks, axis=0).astype(np.float32), res


def kernel(**inputs):
    out, _ = run(trace=False, **inputs)
    return out
